# revision 4
# baseline (speedup 1.0000x reference)
"""DualGNNModel Trainium2 kernel (8 NeuronCores, Bass/Tile) — fp16 pipeline.

Self-contained: accepts FULL inputs (as reference.setup_inputs()), returns the
FULL [256, 1] float32 output.

Sharding: cores 0-3 run the solute GCN encoder, cores 4-7 the solvent encoder.
Within each 4-core group, edges are partitioned by destination node into 4
contiguous 12800-node ranges; GCN/MLP weights are replicated. Per layer each
core:
  dense   b[node] = nsrc[node] * (h @ W)[node]     (fp16 table, 512B rows:
          128 fp16 payload + 256B pad so dma_gather rows hit the 512B
          line-rate; nsrc folded in via the ACT copy scale)
  gather  g = b[src] rows (dma_gather, two int16 half-table passes)
  scatter psum[feat, dstwin] += g_chunk^T @ onehot(dst)   (fp16 PE matmuls,
          window=128 dst)
  epi     hT[:, win] = relu(psum * ndst[win] + bias)      (DVE mult + ACT)
h shards are AllGather'd (fp16) between layers; after layer 3 each core pools
its shard via a membership-one-hot matmul, partial pools are AllReduce'd
within the group, the two encoders' pooled embeddings are exchanged pairwise,
and the (tiny fp32) MLP head runs replicated on every core.
"""
import numpy as np
import concourse.bass as bass
import concourse.bacc as bacc
import concourse.mybir as mybir
import concourse.tile as tile
from concourse.library_config import mlp as mlp_lib
from concourse.masks import make_identity
from concourse.bass_utils import run_bass_kernel_spmd

F32 = mybir.dt.float32
F16 = mybir.dt.float16
I16 = mybir.dt.int16
AF = mybir.ActivationFunctionType
ALU = mybir.AluOpType

CFG = dict(N=50000, E=800000, G=256, DIN=64, DH=128, R=4, NLOC=12800,
           SUPW=1, GMAX=1536, HALF=25600)


def _fill_cfg(cfg):
    c = dict(cfg)
    c["NTOT"] = c["R"] * c["NLOC"]
    c["NW"] = c["NLOC"] // 128
    return c


def _node_norms(cfg, src, dst):
    N = cfg["N"]
    deg_out = np.bincount(src, minlength=N).astype(np.float64)
    deg_in = np.bincount(dst, minlength=N).astype(np.float64)
    nsrc = np.clip(deg_out, 1.0, None) ** -0.5
    ndst = np.clip(deg_in, 1.0, None) ** -0.5
    return nsrc.astype(np.float32), ndst.astype(np.float32)


def _rank_edges(cfg, src, dst, rank):
    NLOC, HALF = cfg["NLOC"], cfg["HALF"]
    lo, hi = rank * NLOC, (rank + 1) * NLOC
    sel = (dst >= lo) & (dst < hi)
    s, d = src[sel], dst[sel]
    order = np.argsort(d, kind="stable")
    s, d = s[order], d[order]
    win = (d - lo) // 128
    half = s // HALF
    out = {}
    for wi in np.unique(win):
        m = win == wi
        for h in (0, 1):
            mh = m & (half == h)
            if mh.any():
                out[(int(wi), h)] = (s[mh],
                                     (d[mh] - lo - wi * 128).astype(np.float32))
    return out


def _build_schedule(cfg, per_core_edges):
    """Cells are (super-window, half): the SUPW consecutive windows' edges are
    packed contiguously (dst_rel relative to the super-window base) and padded
    once per cell. Each super-window accumulates into one PSUM tile
    [128, SUPW*128]."""
    NW, SUPW, GMAX = cfg["NW"], cfg["SUPW"], cfg["GMAX"]
    nsw = (NW + SUPW - 1) // SUPW
    nch_sh = {}
    for si in range(nsw):
        wins = range(si * SUPW, min((si + 1) * SUPW, NW))
        for h in (0, 1):
            mx = 0
            for pc in per_core_edges:
                tot = sum(len(pc[(w, h)][0]) for w in wins if (w, h) in pc)
                mx = max(mx, (tot + 127) // 128)
            if mx:
                nch_sh[(si, h)] = mx

    chunk_sw = []
    sw_instrs = []
    pos = 0
    for si in range(nsw):
        il = []
        for h in (0, 1):
            k = nch_sh.get((si, h), 0)
            chunk_sw.extend([si] * k)
            run_start = pos
            pos += k * 128
            st = run_start
            while st < pos:
                n = min(GMAX, pos - st)
                il.append((h, st, n))
                st += n
        sw_instrs.append(il)
    rows = pos
    chunk_sw = np.asarray(chunk_sw, np.int64)
    last_chunk = np.full(nsw, -1, np.int64)
    for c, si in enumerate(chunk_sw):
        last_chunk[si] = c
    assert (last_chunk >= 0).all(), "empty super-window"

    per_core = []
    for pc in per_core_edges:
        gsrc = np.zeros(rows, np.int64)
        drel = np.full(rows, -1.0, np.float32)
        p = 0
        for si in range(nsw):
            wins = range(si * SUPW, min((si + 1) * SUPW, NW))
            for h in (0, 1):
                k = nch_sh.get((si, h), 0)
                if not k:
                    continue
                cell_end = p + k * 128
                for wi in wins:
                    if (wi, h) in pc:
                        s, dr = pc[(wi, h)]
                        n = len(s)
                        gsrc[p:p + n] = s
                        drel[p:p + n] = dr + (wi - si * SUPW) * 128
                        p += n
                assert p <= cell_end
                p = cell_end
        assert p == rows
        per_core.append(dict(gsrc=gsrc, drel=drel))
    sched = dict(rows=rows, chunk_sw=chunk_sw, sw_instrs=sw_instrs,
                 last_chunk=last_chunk)
    return sched, per_core


def _wrap_idx16(gsrc, half_size):
    rows = len(gsrc)
    rel = (gsrc % half_size).astype(np.int16)
    blk = rel.reshape(rows // 16, 16).T
    return np.tile(blk, (8, 1)).copy()


def _mat128(vec):
    rows = len(vec)
    return np.ascontiguousarray(vec.reshape(rows // 128, 128).T)


def _build_nc(cfg, sched, b2_const, n_cores, nrep=1):
    N, E, G, DIN, DH, R, NLOC, NW, SUPW, GMAX, HALF, NTOT = (
        cfg[k] for k in ("N", "E", "G", "DIN", "DH", "R", "NLOC", "NW",
                         "SUPW", "GMAX", "HALF", "NTOT"))
    WSW = SUPW * 128
    ROWS = sched["rows"]
    NCH = ROWS // 128
    last_chunk = sched["last_chunk"]
    sw_instrs = sched["sw_instrs"]

    nc = bacc.Bacc("TRN2", target_bir_lowering=False, debug=False,
                   enable_asserts=True, num_devices=n_cores)

    def dram(name, shape, dt=F32, kind="ExternalInput"):
        return nc.dram_tensor(name, shape, dt, kind=kind).ap()

    xT = dram("xT", [DIN, NTOT], F16)
    gidx = dram("gidx", [128, ROWS // 16], I16)
    drmat = dram("drmat", [128, NCH], F16)
    iota = dram("iota", [128, WSW], F16)
    iotaG = dram("iotaG", [128, G], F16)
    gidrow = dram("gidrow", [128, NW], F16)
    nsrcm = dram("nsrcm", [128, NTOT // 128], F32)
    vndst = dram("vndst", [128, NLOC], F16)
    W0 = dram("W0", [DIN, DH], F16)
    W1 = dram("W1", [DH, DH], F16)
    W2 = dram("W2", [DH, DH], F16)
    bcol = dram("bcol", [DH, 3], F32)
    mW0su = dram("mW0su", [DH, 128])
    mW0sv = dram("mW0sv", [DH, 128])
    mW0gf = dram("mW0gf", [4, 128])
    mW1 = dram("mW1", [128, 64])
    mW2 = dram("mW2", [64, 1])
    b0c = dram("b0c", [128, 1])
    b1c = dram("b1c", [64, 1])
    gfT = dram("gfT", [4, G])
    icnt_su = dram("icnt_su", [128, G])
    icnt_sv = dram("icnt_sv", [128, G])
    y = dram("y", [G, 1], kind="ExternalOutput")

    with tile.TileContext(nc) as tc:
        with tc.tile_pool(name="const", bufs=1) as cpool, \
             tc.tile_pool(name="hT", bufs=1) as hpool, \
             tc.tile_pool(name="gath", bufs=4) as gpool, \
             tc.tile_pool(name="oneh", bufs=2) as opool, \
             tc.tile_pool(name="dense", bufs=4) as dpool, \
             tc.tile_pool(name="epi", bufs=2) as epool, \
             tc.tile_pool(name="psc", bufs=4, space="PSUM") as psc, \
             tc.tile_pool(name="psd", bufs=4, space="PSUM") as psd, \
             tc.tile_pool(name="hd", bufs=1) as hdpool, \
             tc.tile_pool(name="dram", bufs=1, space="DRAM") as drp:

            nc.gpsimd.load_library(mlp_lib)

            t_gidx = cpool.tile([128, ROWS // 16], I16)
            nc.sync.dma_start(out=t_gidx[:], in_=gidx[:])
            t_dr = cpool.tile([128, NCH], F16)
            nc.sync.dma_start(out=t_dr[:], in_=drmat[:])
            t_iota = cpool.tile([128, WSW], F16)
            nc.sync.dma_start(out=t_iota[:], in_=iota[:])
            t_iotaG = cpool.tile([128, G], F16)
            nc.sync.dma_start(out=t_iotaG[:], in_=iotaG[:])
            t_gidrow = cpool.tile([128, NW], F16)
            nc.sync.dma_start(out=t_gidrow[:], in_=gidrow[:])
            t_nsrc = cpool.tile([128, NTOT // 128], F32)
            nc.sync.dma_start(out=t_nsrc[:], in_=nsrcm[:])
            t_vndst = cpool.tile([128, NLOC], F16)
            nc.sync.dma_start(out=t_vndst[:], in_=vndst[:])
            t_bcol = cpool.tile([DH, 3], F32)
            nc.sync.dma_start(out=t_bcol[:], in_=bcol[:])
            t_ident = cpool.tile([128, 128], F16)
            make_identity(nc, t_ident[:])
            t_W = []
            for nm, ap_, k in (("w0", W0, DIN), ("w1", W1, DH), ("w2", W2, DH)):
                tw = cpool.tile([k, DH], F16, name=f"t_{nm}")
                nc.sync.dma_start(out=tw[:], in_=ap_[:])
                t_W.append(tw)

            def one_pass(rep):
                sfx = f"_{rep}"

                t_hT = hpool.tile([128, NLOC], F16)

                # Table rows are 512B (f32-typed for the fast dma_gather
                # ucode path) holding 128 f16 payload + 256B pad; consumers
                # bitcast to f16.
                btbl = [[drp.tile([HALF, 128], F32, name=f"btbl{i}h{h}")
                         for h in range(2)] for i in range(2)]
                # NSEG segments of the h AllGather overlap collective wire
                # time with the scatter phase.
                NSEG = 5
                SEGW = NW // NSEG
                SEGN = SEGW * 128
                cinq = [drp.tile([128, SEGN], F16, name=f"cin{q}" + sfx)
                        for q in range(NSEG)]
                hTall = [[drp.tile([R, 128, SEGN], F16, name=f"hTall{i}q{q}" + sfx)
                          for q in range(NSEG)] for i in range(2)]
                pool_cin = drp.tile([128, G], F32, name="pool_cin")
                pool_out = drp.tile([128, G], F32, name="pool_out")
                pair_cin = drp.tile([128, G], F32, name="pair_cin")
                pair_out = drp.tile([2, 128, G], F32, name="pair_out")

                group_a = [list(range(R)), list(range(R, 2 * R))]
                group_pairs = [[r, r + R] for r in range(R)]

                def dense(l):
                    W = t_W[l]
                    K = DIN if l == 0 else DH
                    tbl = btbl[l % 2]
                    for rb in range(R):
                        for c5 in range(NLOC // 512):
                            th = dpool.tile([K, 512], F16, name="th", tag="th")
                            if l == 0:
                                nc.sync.dma_start(
                                    out=th[:],
                                    in_=xT[:, rb * NLOC + c5 * 512:
                                               rb * NLOC + (c5 + 1) * 512])
                            else:
                                q = (c5 * 512) // SEGN
                                off = (c5 * 512) % SEGN
                                nc.sync.dma_start(
                                    out=th[:],
                                    in_=hTall[(l - 1) % 2][q][rb, :, off:off + 512])
                            tb = dpool.tile([128, 4, 128], F16, name="tb", tag="tb")
                            gbase = rb * NLOC + c5 * 512
                            for j in range(4):
                                pd = psd.tile([128, 128], F32, name="pd", tag="pd")
                                nc.tensor.matmul(out=pd[:],
                                                 lhsT=th[:, j * 128:(j + 1) * 128],
                                                 rhs=W[:], start=True, stop=True)
                                gc = gbase // 128 + j
                                nc.scalar.activation(out=tb[:, j, :], in_=pd[:],
                                                     func=AF.Copy,
                                                     scale=t_nsrc[:, gc:gc + 1])
                            tb_h = tbl[gbase // HALF]
                            base = gbase % HALF
                            nc.sync.dma_start(
                                out=tb_h.tensor.ap().bitcast(F16)
                                    [base:base + 512, 0:128]
                                    .rearrange("(c p) d -> p c d", p=128),
                                in_=tb[:])

                def scatter(l, do_ag):
                    tbl = btbl[l % 2]
                    for si in range(NW):
                        ps = psc.tile([128, WSW], F32, name="ps", tag="pw")
                        first = True
                        for (half, st, n) in sw_instrs[si]:
                            k = n // 128
                            tg = gpool.tile([128, GMAX // 128, 128], F32,
                                            name="tg", tag="tg")
                            nc.gpsimd.dma_gather(
                                out_ap=tg[:, :k, :],
                                in_ap=tbl[half].tensor.ap()[:],
                                idxs_ap=t_gidx[:, st // 16:(st + n) // 16],
                                num_idxs=n, num_idxs_reg=n, elem_size=128,
                                single_packet=False)
                            toh = opool.tile([128, GMAX // 128, WSW], F16,
                                             name="toh", tag="toh")
                            nc.vector.tensor_tensor(
                                out=toh[:, :k, :],
                                in0=t_dr[:, st // 128:st // 128 + k, None]
                                    .to_broadcast([128, k, WSW]),
                                in1=t_iota[:, None, :].to_broadcast([128, k, WSW]),
                                op=ALU.is_equal)
                            for j in range(k):
                                ch = st // 128 + j
                                nc.tensor.matmul(out=ps[:],
                                                 lhsT=tg[:, j, :]
                                                     .bitcast(F16)[:, 0:128],
                                                 rhs=toh[:, j, :], start=first,
                                                 stop=bool(ch == last_chunk[si]))
                                first = False
                        t_epi = epool.tile([128, WSW], F16, name="t_epi",
                                           tag="t_epi")
                        nc.vector.tensor_tensor(
                            out=t_epi[:], in0=ps[:],
                            in1=t_vndst[:, si * WSW:(si + 1) * WSW],
                            op=ALU.mult)
                        nc.scalar.activation(
                            out=t_hT[:, si * WSW:(si + 1) * WSW], in_=t_epi[:],
                            func=AF.Relu, bias=t_bcol[:, l:l + 1])
                        if do_ag and ((si + 1) * WSW) % SEGN == 0:
                            q = ((si + 1) * WSW) // SEGN - 1
                            nc.sync.dma_start(
                                out=cinq[q][:],
                                in_=t_hT[:, q * SEGN:(q + 1) * SEGN])
                            nc.gpsimd.collective_compute(
                                "AllGather", ALU.bypass,
                                replica_groups=group_a,
                                ins=[cinq[q][:]],
                                outs=[hTall[l % 2][q][:]])
                        if not do_ag:
                            # layer 3: pool this window now
                            ptr = psd.tile([128, 128], F16,
                                           name="ptr" + sfx, tag="pd")
                            nc.tensor.transpose(
                                out=ptr[:],
                                in_=t_hT[:, si * 128:(si + 1) * 128],
                                identity=t_ident[:])
                            t_hrow = dpool.tile([128, 128], F16,
                                                name="t_hrow" + sfx, tag="th")
                            nc.scalar.activation(out=t_hrow[:], in_=ptr[:],
                                                 func=AF.Copy)
                            t_memb = dpool.tile([128, G], F16,
                                                name="t_memb" + sfx, tag="tb")
                            nc.vector.tensor_tensor(
                                out=t_memb[:],
                                in0=t_gidrow[:, si:si + 1]
                                    .to_broadcast([128, G]),
                                in1=t_iotaG[:], op=ALU.is_equal)
                            nc.tensor.matmul(
                                out=ppool[:], lhsT=t_hrow[:], rhs=t_memb[:],
                                start=si == 0, stop=si == NW - 1)

                ppool = psd.tile([128, G], F32, name="ppool" + sfx, tag="pd")
                for l in range(3):
                    dense(l)
                    scatter(l, do_ag=l < 2)

                t_pool = hdpool.tile([128, G], F32, name="t_pool" + sfx, tag="t_pool")
                nc.scalar.activation(out=t_pool[:], in_=ppool[:], func=AF.Copy)
                nc.sync.dma_start(out=pool_cin[:], in_=t_pool[:])
                nc.gpsimd.collective_compute(
                    "AllReduce", ALU.add, replica_groups=group_a,
                    ins=[pool_cin[:]], outs=[pool_out[:]])
                t_pool2 = hdpool.tile([128, G], F32, name="t_pool2" + sfx, tag="t_pool2")
                nc.sync.dma_start(out=t_pool2[:], in_=pool_out[:])
                nc.sync.dma_start(out=pair_cin[:], in_=t_pool2[:])
                nc.gpsimd.collective_compute(
                    "AllGather", ALU.bypass, replica_groups=group_pairs,
                    ins=[pair_cin[:]], outs=[pair_out[:]])

                t_su = hdpool.tile([128, G], F32, name="t_su" + sfx, tag="t_su")
                t_sv = hdpool.tile([128, G], F32, name="t_sv" + sfx, tag="t_sv")
                t_icsu = hdpool.tile([128, G], F32, name="t_icsu" + sfx, tag="t_icsu")
                nc.sync.dma_start(out=t_icsu[:], in_=icnt_su[:])
                t_icsv = hdpool.tile([128, G], F32, name="t_icsv" + sfx, tag="t_icsv")
                nc.sync.dma_start(out=t_icsv[:], in_=icnt_sv[:])
                t_su_raw = hdpool.tile([128, G], F32, name="t_su_raw" + sfx, tag="t_su_raw")
                nc.sync.dma_start(out=t_su_raw[:], in_=pair_out[0])
                t_sv_raw = hdpool.tile([128, G], F32, name="t_sv_raw" + sfx, tag="t_sv_raw")
                nc.sync.dma_start(out=t_sv_raw[:], in_=pair_out[1])
                nc.vector.tensor_tensor(out=t_su[:], in0=t_su_raw[:], in1=t_icsu[:],
                                        op=ALU.mult)
                nc.vector.tensor_tensor(out=t_sv[:], in0=t_sv_raw[:], in1=t_icsv[:],
                                        op=ALU.mult)
                t_gf = hdpool.tile([4, G], F32, name="t_gf" + sfx, tag="t_gf")
                nc.sync.dma_start(out=t_gf[:], in_=gfT[:])
                t_mW0su = hdpool.tile([DH, 128], F32, name="t_mW0su" + sfx, tag="t_mW0su")
                nc.sync.dma_start(out=t_mW0su[:], in_=mW0su[:])
                t_mW0sv = hdpool.tile([DH, 128], F32, name="t_mW0sv" + sfx, tag="t_mW0sv")
                nc.sync.dma_start(out=t_mW0sv[:], in_=mW0sv[:])
                t_mW0gf = hdpool.tile([4, 128], F32, name="t_mW0gf" + sfx, tag="t_mW0gf")
                nc.sync.dma_start(out=t_mW0gf[:], in_=mW0gf[:])
                t_mW1 = hdpool.tile([128, 64], F32, name="t_mW1" + sfx, tag="t_mW1")
                nc.sync.dma_start(out=t_mW1[:], in_=mW1[:])
                t_mW2 = hdpool.tile([64, 1], F32, name="t_mW2" + sfx, tag="t_mW2")
                nc.sync.dma_start(out=t_mW2[:], in_=mW2[:])
                t_b0c = hdpool.tile([128, 1], F32, name="t_b0c" + sfx, tag="t_b0c")
                nc.sync.dma_start(out=t_b0c[:], in_=b0c[:])
                t_b1c = hdpool.tile([64, 1], F32, name="t_b1c" + sfx, tag="t_b1c")
                nc.sync.dma_start(out=t_b1c[:], in_=b1c[:])

                ph1 = psd.tile([128, G], F32, name="ph1" + sfx, tag="pd")
                nc.tensor.matmul(out=ph1[:], lhsT=t_mW0su[:], rhs=t_su[:],
                                 start=True, stop=False)
                nc.tensor.matmul(out=ph1[:], lhsT=t_mW0sv[:], rhs=t_sv[:],
                                 start=False, stop=False)
                nc.tensor.matmul(out=ph1[:], lhsT=t_mW0gf[:], rhs=t_gf[:],
                                 start=False, stop=True)
                t_h1 = hdpool.tile([128, G], F32, name="t_h1" + sfx, tag="t_h1")
                nc.scalar.activation(out=t_h1[:], in_=ph1[:], func=AF.Relu,
                                     bias=t_b0c[:, :1])
                ph2 = psd.tile([64, G], F32, name="ph2" + sfx, tag="pd")
                nc.tensor.matmul(out=ph2[:], lhsT=t_mW1[:], rhs=t_h1[:],
                                 start=True, stop=True)
                t_h2 = hdpool.tile([64, G], F32, name="t_h2" + sfx, tag="t_h2")
                nc.scalar.activation(out=t_h2[:], in_=ph2[:], func=AF.Relu,
                                     bias=t_b1c[:, :1])
                po = psd.tile([1, G], F32, name="po" + sfx, tag="pd")
                nc.tensor.matmul(out=po[:], lhsT=t_mW2[:], rhs=t_h2[:],
                                 start=True, stop=True)
                t_o = hdpool.tile([1, G], F32, name="t_o" + sfx, tag="t_o")
                nc.scalar.activation(out=t_o[:], in_=po[:], func=AF.Copy,
                                     bias=float(b2_const))
                nc.sync.dma_start(out=y[:], in_=t_o[:, :, None])

            for rep in range(nrep):
                one_pass(rep)

    nc.compile()
    return nc


def _host_prep(cfg, inputs):
    cfg = _fill_cfg(cfg)
    N, G, DIN, DH, R, NLOC, NW, NTOT, HALF = (
        cfg[k] for k in ("N", "G", "DIN", "DH", "R", "NLOC", "NW", "NTOT",
                         "HALF"))
    enc = []
    for pre in ("solute", "solvent"):
        src = np.asarray(inputs[f"{pre}_src"]).astype(np.int64)
        dst = np.asarray(inputs[f"{pre}_dst"]).astype(np.int64)
        gid = np.asarray(inputs[f"{pre}_gid"]).astype(np.int64)
        x = np.asarray(inputs[f"{pre}_x"], np.float32)
        nsrc, ndst = _node_norms(cfg, src, dst)
        enc.append(dict(src=src, dst=dst, gid=gid, x=x, nsrc=nsrc, ndst=ndst))

    per_core_edges = []
    for e in enc:
        for r in range(R):
            per_core_edges.append(_rank_edges(cfg, e["src"], e["dst"], r))
    sched, pc_arrays = _build_schedule(cfg, per_core_edges)

    iota = np.broadcast_to(np.arange(cfg["SUPW"] * 128, dtype=np.float16),
                           (128, cfg["SUPW"] * 128)).copy()
    iotaG = np.broadcast_to(np.arange(G, dtype=np.float16), (128, G)).copy()
    gfT = np.ascontiguousarray(np.asarray(inputs["global_feats"], np.float32).T)
    mW0 = np.asarray(inputs["mlp_W0"], np.float32)
    icnts = []
    for e in enc:
        cnt = np.maximum(np.bincount(e["gid"], minlength=G), 1.0).astype(np.float32)
        icnts.append(np.broadcast_to(1.0 / cnt, (128, G)).copy())
    b2_const = float(np.asarray(inputs["mlp_b2"]).reshape(-1)[0])

    xTs, gidrows, nsrcms, vndsts = [], [], [], []
    for e in enc:
        xp = np.zeros((NTOT, DIN), np.float16)
        xp[:N] = e["x"].astype(np.float16)
        xTs.append(np.ascontiguousarray(xp.T))
        gr = np.full(NTOT, -1.0, np.float16)
        gr[:N] = e["gid"].astype(np.float16)
        gidrows.append(gr)
        nsp = np.zeros(NTOT, np.float32)
        nsp[:N] = e["nsrc"]
        nsrcms.append(_mat128(nsp))
        ndp = np.zeros(NTOT, np.float16)
        ndp[:N] = e["ndst"].astype(np.float16)
        vndsts.append(ndp)

    in_maps = []
    for gi in range(2):
        pre = "su" if gi == 0 else "sv"
        for r in range(R):
            c = gi * R + r
            arr = pc_arrays[c]
            gr_loc = gidrows[gi][r * NLOC:(r + 1) * NLOC]
            vnd_loc = np.broadcast_to(vndsts[gi][r * NLOC:(r + 1) * NLOC],
                                      (128, NLOC)).copy()
            im = dict(
                xT=xTs[gi],
                gidx=_wrap_idx16(arr["gsrc"], HALF),
                drmat=_mat128(arr["drel"]).astype(np.float16),
                iota=iota, iotaG=iotaG,
                gidrow=_mat128(gr_loc),
                nsrcm=nsrcms[gi],
                vndst=vnd_loc,
                W0=np.asarray(inputs[f"{pre}_W0"], np.float32).astype(np.float16),
                W1=np.asarray(inputs[f"{pre}_W1"], np.float32).astype(np.float16),
                W2=np.asarray(inputs[f"{pre}_W2"], np.float32).astype(np.float16),
                bcol=np.ascontiguousarray(
                    np.asarray(inputs[f"{pre}_b"], np.float32).T),
                mW0su=np.ascontiguousarray(mW0[0:DH, :]),
                mW0sv=np.ascontiguousarray(mW0[DH:2 * DH, :]),
                mW0gf=np.ascontiguousarray(mW0[2 * DH:2 * DH + 4, :]),
                mW1=np.asarray(inputs["mlp_W1"], np.float32),
                mW2=np.asarray(inputs["mlp_W2"], np.float32),
                b0c=np.asarray(inputs["mlp_b0"], np.float32).reshape(128, 1),
                b1c=np.asarray(inputs["mlp_b1"], np.float32).reshape(64, 1),
                gfT=gfT, icnt_su=icnts[0], icnt_sv=icnts[1],
            )
            in_maps.append(im)
    return cfg, sched, b2_const, in_maps


_CACHE = {}


def kernel(**inputs) -> np.ndarray:
    cfg, sched, b2c, in_maps = _host_prep(CFG, inputs)
    key = (sched["rows"], b2c, sched["chunk_sw"].tobytes(),
           tuple(i for sw in sched["sw_instrs"] for i in sw))
    nc = _CACHE.get(key)
    if nc is None:
        nc = _build_nc(cfg, sched, b2c, 8)
        _CACHE[key] = nc
    res = run_bass_kernel_spmd(nc, in_maps, core_ids=list(range(8)))
    return np.asarray(res.results[0]["y"], np.float32)


# revision 12
# speedup vs baseline: 1.1796x; 1.1796x over previous
"""DualGNNModel Trainium2 kernel (8 NeuronCores, Bass/Tile) — fp16 pipeline.

Self-contained: accepts FULL inputs (as reference.setup_inputs()), returns the
FULL [256, 1] float32 output.

Sharding: cores 0-3 run the solute GCN encoder, cores 4-7 the solvent encoder.
Within each 4-core group, edges are partitioned by destination node into 4
contiguous 12800-node ranges; GCN/MLP weights are replicated. Per layer each
core:
  dense   b[node] = nsrc[node] * (h @ W)[node]     (fp16 table, 512B rows:
          128 fp16 payload + 256B pad so dma_gather rows hit the 512B
          line-rate; nsrc folded in via the ACT copy scale)
  gather  g = b[src] rows (dma_gather, two int16 half-table passes)
  scatter psum[feat, dstwin] += g_chunk^T @ onehot(dst)   (fp16 PE matmuls,
          window=128 dst)
  epi     hT[:, win] = relu(psum * ndst[win] + bias)      (DVE mult + ACT)
h shards are AllGather'd (fp16) between layers; after layer 3 each core pools
its shard via a membership-one-hot matmul, partial pools are AllReduce'd
within the group, the two encoders' pooled embeddings are exchanged pairwise,
and the (tiny fp32) MLP head runs replicated on every core.
"""
import numpy as np
import concourse.bass as bass
import concourse.bacc as bacc
import concourse.mybir as mybir
import concourse.tile as tile
from concourse.library_config import mlp as mlp_lib
from concourse.masks import make_identity
from concourse.bass_utils import run_bass_kernel_spmd

F32 = mybir.dt.float32
F16 = mybir.dt.float16
I16 = mybir.dt.int16
AF = mybir.ActivationFunctionType
ALU = mybir.AluOpType

CFG = dict(N=50000, E=800000, G=256, DIN=64, DH=128, R=4, NLOC=12800,
           SUPW=1, GMAX=1536, HALF=25600)


def _fill_cfg(cfg):
    c = dict(cfg)
    c["NTOT"] = c["R"] * c["NLOC"]
    c["NW"] = c["NLOC"] // 128
    return c


def _node_norms(cfg, src, dst):
    N = cfg["N"]
    deg_out = np.bincount(src, minlength=N).astype(np.float64)
    deg_in = np.bincount(dst, minlength=N).astype(np.float64)
    nsrc = np.clip(deg_out, 1.0, None) ** -0.5
    ndst = np.clip(deg_in, 1.0, None) ** -0.5
    return nsrc.astype(np.float32), ndst.astype(np.float32)


def _pack_half(cc0, cc1, cap0, cap1, NW, margin=10.0):
    """Fill 2*NW bins of 128 nodes; steer each bin's (sum c0, sum c1) toward
    (cap0, cap1) - margin via degree-bucket greedy."""
    nb = 2 * NW
    caps0 = np.concatenate([cap0, cap0]).astype(float) - margin
    caps1 = np.concatenate([cap1, cap1]).astype(float) - margin
    n = len(cc0)
    bucket = {}
    for i in range(n):
        bucket.setdefault((int(cc0[i]), int(cc1[i])), []).append(i)
    keys = np.array(list(bucket.keys()), dtype=float)
    kcnt = np.array([len(bucket[(int(k[0]), int(k[1]))]) for k in keys], float)
    binof = np.full(n, -1, np.int32)
    for b in range(nb):
        t0, t1 = caps0[b], caps1[b]
        c0s = c1s = 0.0
        for step in range(128):
            rem = 128 - step
            d0 = (t0 - c0s) / rem
            d1 = (t1 - c1s) / rem
            score = (keys[:, 0] - d0) ** 2 + (keys[:, 1] - d1) ** 2
            score[kcnt <= 0] = np.inf
            kidx = int(np.argmin(score))
            k = (int(keys[kidx, 0]), int(keys[kidx, 1]))
            i = bucket[k].pop()
            kcnt[kidx] -= 1
            binof[i] = b
            c0s += k[0]; c1s += k[1]
    return binof


def _mk_targets(M, NW, margin):
    need = int(np.ceil((M + 2 * NW * margin) / 128)) + 1
    base = need // NW
    extra = need - base * NW
    k = np.full(NW, base)
    k[:extra] = base + 1
    return k


def _balance_perms(cfg, enc_edges, margin=10.0):
    """Per-encoder node permutation equalizing per-(window, src-half) edge
    counts across cores so schedule cells pack to 128-multiples."""
    NTOT, HALF, NLOC, NW = (cfg[k] for k in ("NTOT", "HALF", "NLOC", "NW"))
    cs = []
    for (src, dst) in enc_edges:
        c0 = np.bincount(dst[src < HALF], minlength=NTOT)
        c1 = np.bincount(dst[src >= HALF], minlength=NTOT)
        cs.append((c0, c1))
    M0 = M1 = 0
    for (c0, c1) in cs:
        for H in (0, 1):
            lo, hi = H * HALF, (H + 1) * HALF
            M0 = max(M0, int(np.ceil(c0[lo:hi].sum() / 2)))
            M1 = max(M1, int(np.ceil(c1[lo:hi].sum() / 2)))
    k0 = _mk_targets(M0, NW, margin)
    k1 = _mk_targets(M1, NW, margin)[::-1].copy()
    cap0 = k0 * 128.0
    cap1 = k1 * 128.0
    pis = []
    for (c0, c1) in cs:
        pi = np.zeros(NTOT, np.int64)
        for H in (0, 1):
            nodes = np.arange(H * HALF, (H + 1) * HALF)
            binof = _pack_half(c0[nodes].astype(float), c1[nodes].astype(float),
                               cap0, cap1, NW, margin)
            slot = np.zeros(2 * NW, np.int32)
            for i in range(len(nodes)):
                b = binof[i]
                core = H * 2 + (b // NW)
                wi = b % NW
                pi[nodes[i]] = core * NLOC + wi * 128 + slot[b]
                slot[b] += 1
        pis.append(pi)
    return pis


def _rank_edges(cfg, src, dst, rank):
    NLOC, HALF = cfg["NLOC"], cfg["HALF"]
    lo, hi = rank * NLOC, (rank + 1) * NLOC
    sel = (dst >= lo) & (dst < hi)
    s, d = src[sel], dst[sel]
    order = np.argsort(d, kind="stable")
    s, d = s[order], d[order]
    win = (d - lo) // 128
    half = s // HALF
    out = {}
    for wi in np.unique(win):
        m = win == wi
        for h in (0, 1):
            mh = m & (half == h)
            if mh.any():
                out[(int(wi), h)] = (s[mh],
                                     (d[mh] - lo - wi * 128).astype(np.float32))
    return out


def _build_schedule(cfg, per_core_edges):
    """Cells are (super-window, half): the SUPW consecutive windows' edges are
    packed contiguously (dst_rel relative to the super-window base) and padded
    once per cell. Each super-window accumulates into one PSUM tile
    [128, SUPW*128]."""
    NW, SUPW, GMAX = cfg["NW"], cfg["SUPW"], cfg["GMAX"]
    nsw = (NW + SUPW - 1) // SUPW
    nch_sh = {}
    for si in range(nsw):
        wins = range(si * SUPW, min((si + 1) * SUPW, NW))
        for h in (0, 1):
            mx = 0
            for pc in per_core_edges:
                tot = sum(len(pc[(w, h)][0]) for w in wins if (w, h) in pc)
                mx = max(mx, (tot + 127) // 128)
            if mx:
                nch_sh[(si, h)] = mx

    chunk_sw = []
    sw_instrs = []
    pos = 0
    for si in range(nsw):
        il = []
        for h in (0, 1):
            k = nch_sh.get((si, h), 0)
            chunk_sw.extend([si] * k)
            run_start = pos
            pos += k * 128
            st = run_start
            while st < pos:
                n = min(GMAX, pos - st)
                il.append((h, st, n))
                st += n
        sw_instrs.append(il)
    rows = pos
    chunk_sw = np.asarray(chunk_sw, np.int64)
    last_chunk = np.full(nsw, -1, np.int64)
    for c, si in enumerate(chunk_sw):
        last_chunk[si] = c
    assert (last_chunk >= 0).all(), "empty super-window"

    per_core = []
    for pc in per_core_edges:
        gsrc = np.zeros(rows, np.int64)
        drel = np.full(rows, -1.0, np.float32)
        p = 0
        for si in range(nsw):
            wins = range(si * SUPW, min((si + 1) * SUPW, NW))
            for h in (0, 1):
                k = nch_sh.get((si, h), 0)
                if not k:
                    continue
                cell_end = p + k * 128
                for wi in wins:
                    if (wi, h) in pc:
                        s, dr = pc[(wi, h)]
                        n = len(s)
                        gsrc[p:p + n] = s
                        drel[p:p + n] = dr + (wi - si * SUPW) * 128
                        p += n
                assert p <= cell_end
                p = cell_end
        assert p == rows
        per_core.append(dict(gsrc=gsrc, drel=drel))
    sched = dict(rows=rows, chunk_sw=chunk_sw, sw_instrs=sw_instrs,
                 last_chunk=last_chunk)
    return sched, per_core


def _wrap_idx16(gsrc, half_size):
    rows = len(gsrc)
    rel = np.where(gsrc < 0, -1, gsrc % half_size).astype(np.int16)
    blk = rel.reshape(rows // 16, 16).T
    return np.tile(blk, (8, 1)).copy()


def _mat128(vec):
    rows = len(vec)
    return np.ascontiguousarray(vec.reshape(rows // 128, 128).T)


def _build_nc(cfg, sched, b2_const, n_cores, nrep=1):
    N, E, G, DIN, DH, R, NLOC, NW, SUPW, GMAX, HALF, NTOT = (
        cfg[k] for k in ("N", "E", "G", "DIN", "DH", "R", "NLOC", "NW",
                         "SUPW", "GMAX", "HALF", "NTOT"))
    WSW = SUPW * 128
    ROWS = sched["rows"]
    NCH = ROWS // 128
    last_chunk = sched["last_chunk"]
    sw_instrs = sched["sw_instrs"]

    nc = bacc.Bacc("TRN2", target_bir_lowering=False, debug=False,
                   enable_asserts=True, num_devices=n_cores)

    def dram(name, shape, dt=F32, kind="ExternalInput"):
        return nc.dram_tensor(name, shape, dt, kind=kind).ap()

    xT = dram("xT", [DIN, NTOT], F16)
    gidx = dram("gidx", [128, ROWS // 16], I16)
    drmat = dram("drmat", [128, NCH], F16)
    iota = dram("iota", [128, WSW], F16)
    iotaG = dram("iotaG", [128, G], F16)
    gidrow = dram("gidrow", [128, NW], F16)
    tohm = dram("tohm", [128, NCH, 128], F16)
    nsrcm = dram("nsrcm", [128, NTOT // 128], F32)
    vndst = dram("vndst", [128, NLOC], F16)
    W0 = dram("W0", [DIN, DH], F16)
    W1 = dram("W1", [DH, DH], F16)
    W2 = dram("W2", [DH, DH], F16)
    bcol = dram("bcol", [DH, 3], F32)
    mW0su = dram("mW0su", [DH, 128])
    mW0sv = dram("mW0sv", [DH, 128])
    mW0gf = dram("mW0gf", [4, 128])
    mW1 = dram("mW1", [128, 64])
    mW2 = dram("mW2", [64, 1])
    b0c = dram("b0c", [128, 1])
    b1c = dram("b1c", [64, 1])
    gfT = dram("gfT", [4, G])
    icnt_su = dram("icnt_su", [128, G])
    icnt_sv = dram("icnt_sv", [128, G])
    y = dram("y", [G, 1], kind="ExternalOutput")

    GB = 8 if "tune1" in ABLATE else 4
    OB = 4 if "tune1" in ABLATE else 2
    with tile.TileContext(nc) as tc:
        with tc.tile_pool(name="const", bufs=1) as cpool, \
             tc.tile_pool(name="hT", bufs=1) as hpool, \
             tc.tile_pool(name="gath", bufs=GB) as gpool, \
             tc.tile_pool(name="oneh", bufs=OB) as opool, \
             tc.tile_pool(name="dense", bufs=4) as dpool, \
             tc.tile_pool(name="epi", bufs=2) as epool, \
             tc.tile_pool(name="psc", bufs=4, space="PSUM") as psc, \
             tc.tile_pool(name="psd", bufs=4, space="PSUM") as psd, \
             tc.tile_pool(name="hd", bufs=1) as hdpool, \
             tc.tile_pool(name="dram", bufs=1, space="DRAM") as drp:

            nc.gpsimd.load_library(mlp_lib)

            t_gidx = cpool.tile([128, ROWS // 16], I16)
            nc.sync.dma_start(out=t_gidx[:], in_=gidx[:])
            t_dr = cpool.tile([128, NCH], F16)
            nc.sync.dma_start(out=t_dr[:], in_=drmat[:])
            t_iota = cpool.tile([128, WSW], F16)
            nc.sync.dma_start(out=t_iota[:], in_=iota[:])
            t_iotaG = cpool.tile([128, G], F16)
            nc.sync.dma_start(out=t_iotaG[:], in_=iotaG[:])
            t_gidrow = cpool.tile([128, NW], F16)
            nc.sync.dma_start(out=t_gidrow[:], in_=gidrow[:])
            t_nsrc = cpool.tile([128, NTOT // 128], F32)
            nc.sync.dma_start(out=t_nsrc[:], in_=nsrcm[:])
            t_vndst = cpool.tile([128, NLOC], F16)
            nc.sync.dma_start(out=t_vndst[:], in_=vndst[:])
            t_bcol = cpool.tile([DH, 3], F32)
            nc.sync.dma_start(out=t_bcol[:], in_=bcol[:])
            t_ident = cpool.tile([128, 128], F16)
            make_identity(nc, t_ident[:])
            t_W = []
            for nm, ap_, k in (("w0", W0, DIN), ("w1", W1, DH), ("w2", W2, DH)):
                tw = cpool.tile([k, DH], F16, name=f"t_{nm}")
                nc.sync.dma_start(out=tw[:], in_=ap_[:])
                t_W.append(tw)

            def one_pass(rep):
                sfx = f"_{rep}"

                t_hT = hpool.tile([128, NLOC], F16)

                # Table rows are 512B (f32-typed for the fast dma_gather
                # ucode path) holding 128 f16 payload + 256B pad; consumers
                # bitcast to f16.
                btbl = [[drp.tile([HALF, 128], F32, name=f"btbl{i}h{h}")
                         for h in range(2)] for i in range(2)]
                # NSEG segments of the h AllGather overlap collective wire
                # time with the scatter phase.
                NSEG = 10 if "ns10" in ABLATE else 5
                SEGW = NW // NSEG
                SEGN = SEGW * 128
                cinq = [drp.tile([128, SEGN], F16, name=f"cin{q}" + sfx)
                        for q in range(NSEG)]
                adsp = "Shared" if "shared_out" in ABLATE else "Local"
                hTall = [[drp.tile([R, 128, SEGN], F16, name=f"hTall{i}q{q}" + sfx,
                                   addr_space=adsp)
                          for q in range(NSEG)] for i in range(2)]
                pool_cin = drp.tile([128, G], F32, name="pool_cin")
                pool_out = drp.tile([128, G], F32, name="pool_out",
                                    addr_space=adsp)
                pair_cin = drp.tile([128, G], F32, name="pair_cin")
                pair_out = drp.tile([2, 128, G], F32, name="pair_out",
                                    addr_space=adsp)

                group_a = [list(range(R)), list(range(R, 2 * R))]
                group_pairs = [[r, r + R] for r in range(R)]

                def dense(l):
                    if "dense" in ABLATE:
                        return
                    W = t_W[l]
                    K = DIN if l == 0 else DH
                    tbl = btbl[l % 2]
                    for rb in range(R):
                        for c5 in range(NLOC // 512):
                            th = dpool.tile([K, 512], F16, name="th", tag="th")
                            if l == 0:
                                nc.sync.dma_start(
                                    out=th[:],
                                    in_=xT[:, rb * NLOC + c5 * 512:
                                               rb * NLOC + (c5 + 1) * 512])
                            else:
                                q = (c5 * 512) // SEGN
                                off = (c5 * 512) % SEGN
                                nc.sync.dma_start(
                                    out=th[:],
                                    in_=hTall[(l - 1) % 2][q][rb, :, off:off + 512])
                            tb = dpool.tile([128, 4, 128], F16, name="tb", tag="tb")
                            gbase = rb * NLOC + c5 * 512
                            for j in range(4):
                                pd = psd.tile([128, 128], F32, name="pd", tag="pd")
                                nc.tensor.matmul(out=pd[:],
                                                 lhsT=th[:, j * 128:(j + 1) * 128],
                                                 rhs=W[:], start=True, stop=True)
                                gc = gbase // 128 + j
                                nc.scalar.activation(out=tb[:, j, :], in_=pd[:],
                                                     func=AF.Copy,
                                                     scale=t_nsrc[:, gc:gc + 1])
                            tb_h = tbl[gbase // HALF]
                            base = gbase % HALF
                            nc.sync.dma_start(
                                out=tb_h.tensor.ap().bitcast(F16)
                                    [base:base + 512, 0:128]
                                    .rearrange("(c p) d -> p c d", p=128),
                                in_=tb[:])

                def scatter(l, do_ag):
                    tbl = btbl[l % 2]
                    if ABLATE & {"scatter", "onehot", "matmul"}:
                        nc.vector.memset(t_hT[:], 0.01)
                    for si in range(NW):
                        ps = psc.tile([128, WSW], F32, name="ps", tag="pw")
                        first = True
                        do_sc = "scatter" not in ABLATE
                        do_oh = do_sc and "onehot" not in ABLATE
                        do_mm = do_oh and "matmul" not in ABLATE
                        for (half, st, n) in (sw_instrs[si] if do_sc else []):
                            k = n // 128
                            tg = gpool.tile([128, GMAX // 128, 128], F32,
                                            name="tg", tag="tg")
                            nc.gpsimd.dma_gather(
                                out_ap=tg[:, :k, :],
                                in_ap=tbl[half].tensor.ap()[:],
                                idxs_ap=t_gidx[:, st // 16:(st + n) // 16],
                                num_idxs=n, num_idxs_reg=n, elem_size=128,
                                single_packet=False)
                            if not do_oh:
                                continue
                            toh = opool.tile([128, GMAX // 128, WSW], F16,
                                             name="toh", tag="toh")
                            if "toh_dram" in ABLATE:
                                nc.sync.dma_start(
                                    out=toh[:, :k, :],
                                    in_=tohm[:, st // 128:st // 128 + k, :])
                            else:
                                nc.vector.tensor_tensor(
                                    out=toh[:, :k, :],
                                    in0=t_dr[:, st // 128:st // 128 + k, None]
                                        .to_broadcast([128, k, WSW]),
                                    in1=t_iota[:, None, :]
                                        .to_broadcast([128, k, WSW]),
                                    op=ALU.is_equal)
                            if not do_mm:
                                continue
                            for j in range(k):
                                ch = st // 128 + j
                                nc.tensor.matmul(out=ps[:],
                                                 lhsT=tg[:, j, :]
                                                     .bitcast(F16)[:, 0:128],
                                                 rhs=toh[:, j, :], start=first,
                                                 stop=bool(ch == last_chunk[si]))
                                first = False
                        if do_mm:
                            t_epi = epool.tile([128, WSW], F16, name="t_epi",
                                               tag="t_epi")
                            nc.vector.tensor_tensor(
                                out=t_epi[:], in0=ps[:],
                                in1=t_vndst[:, si * WSW:(si + 1) * WSW],
                                op=ALU.mult)
                            nc.scalar.activation(
                                out=t_hT[:, si * WSW:(si + 1) * WSW], in_=t_epi[:],
                                func=AF.Relu, bias=t_bcol[:, l:l + 1])
                        if do_ag and ((si + 1) * WSW) % SEGN == 0:
                            q = ((si + 1) * WSW) // SEGN - 1
                            nc.sync.dma_start(
                                out=cinq[q][:],
                                in_=t_hT[:, q * SEGN:(q + 1) * SEGN])
                            if "ag_local" in ABLATE:
                                for r_ in range(R):
                                    nc.sync.dma_start(
                                        out=hTall[l % 2][q][r_],
                                        in_=cinq[q][:])
                            else:
                                nc.gpsimd.collective_compute(
                                    "AllGather", ALU.bypass,
                                    replica_groups=group_a,
                                    ins=[cinq[q][:]],
                                    outs=[hTall[l % 2][q][:]])
                        if not do_ag and do_mm:
                            # layer 3: pool this window now
                            ptr = psd.tile([128, 128], F16,
                                           name="ptr" + sfx, tag="pd")
                            nc.tensor.transpose(
                                out=ptr[:],
                                in_=t_hT[:, si * 128:(si + 1) * 128],
                                identity=t_ident[:])
                            t_hrow = dpool.tile([128, 128], F16,
                                                name="t_hrow" + sfx, tag="th")
                            nc.scalar.activation(out=t_hrow[:], in_=ptr[:],
                                                 func=AF.Copy)
                            t_memb = dpool.tile([128, G], F16,
                                                name="t_memb" + sfx, tag="tb")
                            nc.vector.tensor_tensor(
                                out=t_memb[:],
                                in0=t_gidrow[:, si:si + 1]
                                    .to_broadcast([128, G]),
                                in1=t_iotaG[:], op=ALU.is_equal)
                            nc.tensor.matmul(
                                out=ppool[:], lhsT=t_hrow[:], rhs=t_memb[:],
                                start=si == 0, stop=si == NW - 1)

                ppool = psd.tile([128, G], F32, name="ppool" + sfx, tag="pd")
                for l in range(3):
                    dense(l)
                    scatter(l, do_ag=l < 2)

                t_pool = hdpool.tile([128, G], F32, name="t_pool" + sfx, tag="t_pool")
                if "scatter" in ABLATE or "onehot" in ABLATE or "matmul" in ABLATE:
                    nc.vector.memset(t_pool[:], 0.01)
                else:
                    nc.scalar.activation(out=t_pool[:], in_=ppool[:], func=AF.Copy)
                nc.sync.dma_start(out=pool_cin[:], in_=t_pool[:])
                nc.gpsimd.collective_compute(
                    "AllReduce", ALU.add, replica_groups=group_a,
                    ins=[pool_cin[:]], outs=[pool_out[:]])
                t_pool2 = hdpool.tile([128, G], F32, name="t_pool2" + sfx, tag="t_pool2")
                nc.sync.dma_start(out=t_pool2[:], in_=pool_out[:])
                nc.sync.dma_start(out=pair_cin[:], in_=t_pool2[:])
                nc.gpsimd.collective_compute(
                    "AllGather", ALU.bypass, replica_groups=group_pairs,
                    ins=[pair_cin[:]], outs=[pair_out[:]])

                t_su = hdpool.tile([128, G], F32, name="t_su" + sfx, tag="t_su")
                t_sv = hdpool.tile([128, G], F32, name="t_sv" + sfx, tag="t_sv")
                t_icsu = hdpool.tile([128, G], F32, name="t_icsu" + sfx, tag="t_icsu")
                nc.sync.dma_start(out=t_icsu[:], in_=icnt_su[:])
                t_icsv = hdpool.tile([128, G], F32, name="t_icsv" + sfx, tag="t_icsv")
                nc.sync.dma_start(out=t_icsv[:], in_=icnt_sv[:])
                t_su_raw = hdpool.tile([128, G], F32, name="t_su_raw" + sfx, tag="t_su_raw")
                nc.sync.dma_start(out=t_su_raw[:], in_=pair_out[0])
                t_sv_raw = hdpool.tile([128, G], F32, name="t_sv_raw" + sfx, tag="t_sv_raw")
                nc.sync.dma_start(out=t_sv_raw[:], in_=pair_out[1])
                nc.vector.tensor_tensor(out=t_su[:], in0=t_su_raw[:], in1=t_icsu[:],
                                        op=ALU.mult)
                nc.vector.tensor_tensor(out=t_sv[:], in0=t_sv_raw[:], in1=t_icsv[:],
                                        op=ALU.mult)
                t_gf = hdpool.tile([4, G], F32, name="t_gf" + sfx, tag="t_gf")
                nc.sync.dma_start(out=t_gf[:], in_=gfT[:])
                t_mW0su = hdpool.tile([DH, 128], F32, name="t_mW0su" + sfx, tag="t_mW0su")
                nc.sync.dma_start(out=t_mW0su[:], in_=mW0su[:])
                t_mW0sv = hdpool.tile([DH, 128], F32, name="t_mW0sv" + sfx, tag="t_mW0sv")
                nc.sync.dma_start(out=t_mW0sv[:], in_=mW0sv[:])
                t_mW0gf = hdpool.tile([4, 128], F32, name="t_mW0gf" + sfx, tag="t_mW0gf")
                nc.sync.dma_start(out=t_mW0gf[:], in_=mW0gf[:])
                t_mW1 = hdpool.tile([128, 64], F32, name="t_mW1" + sfx, tag="t_mW1")
                nc.sync.dma_start(out=t_mW1[:], in_=mW1[:])
                t_mW2 = hdpool.tile([64, 1], F32, name="t_mW2" + sfx, tag="t_mW2")
                nc.sync.dma_start(out=t_mW2[:], in_=mW2[:])
                t_b0c = hdpool.tile([128, 1], F32, name="t_b0c" + sfx, tag="t_b0c")
                nc.sync.dma_start(out=t_b0c[:], in_=b0c[:])
                t_b1c = hdpool.tile([64, 1], F32, name="t_b1c" + sfx, tag="t_b1c")
                nc.sync.dma_start(out=t_b1c[:], in_=b1c[:])

                ph1 = psd.tile([128, G], F32, name="ph1" + sfx, tag="pd")
                nc.tensor.matmul(out=ph1[:], lhsT=t_mW0su[:], rhs=t_su[:],
                                 start=True, stop=False)
                nc.tensor.matmul(out=ph1[:], lhsT=t_mW0sv[:], rhs=t_sv[:],
                                 start=False, stop=False)
                nc.tensor.matmul(out=ph1[:], lhsT=t_mW0gf[:], rhs=t_gf[:],
                                 start=False, stop=True)
                t_h1 = hdpool.tile([128, G], F32, name="t_h1" + sfx, tag="t_h1")
                nc.scalar.activation(out=t_h1[:], in_=ph1[:], func=AF.Relu,
                                     bias=t_b0c[:, :1])
                ph2 = psd.tile([64, G], F32, name="ph2" + sfx, tag="pd")
                nc.tensor.matmul(out=ph2[:], lhsT=t_mW1[:], rhs=t_h1[:],
                                 start=True, stop=True)
                t_h2 = hdpool.tile([64, G], F32, name="t_h2" + sfx, tag="t_h2")
                nc.scalar.activation(out=t_h2[:], in_=ph2[:], func=AF.Relu,
                                     bias=t_b1c[:, :1])
                po = psd.tile([1, G], F32, name="po" + sfx, tag="pd")
                nc.tensor.matmul(out=po[:], lhsT=t_mW2[:], rhs=t_h2[:],
                                 start=True, stop=True)
                t_o = hdpool.tile([1, G], F32, name="t_o" + sfx, tag="t_o")
                nc.scalar.activation(out=t_o[:], in_=po[:], func=AF.Copy,
                                     bias=float(b2_const))
                nc.sync.dma_start(out=y[:], in_=t_o[:, :, None])

            for rep in range(nrep):
                one_pass(rep)

    nc.compile()
    return nc


def _host_prep(cfg, inputs):
    cfg = _fill_cfg(cfg)
    N, G, DIN, DH, R, NLOC, NW, NTOT, HALF = (
        cfg[k] for k in ("N", "G", "DIN", "DH", "R", "NLOC", "NW", "NTOT",
                         "HALF"))
    enc = []
    for pre in ("solute", "solvent"):
        src = np.asarray(inputs[f"{pre}_src"]).astype(np.int64)
        dst = np.asarray(inputs[f"{pre}_dst"]).astype(np.int64)
        gid = np.asarray(inputs[f"{pre}_gid"]).astype(np.int64)
        x = np.asarray(inputs[f"{pre}_x"], np.float32)
        nsrc, ndst = _node_norms(cfg, src, dst)
        enc.append(dict(src=src, dst=dst, gid=gid, x=x, nsrc=nsrc, ndst=ndst))
    # node relabeling balances per-(window, src-half) edge counts across cores
    pis = _balance_perms(cfg, [(e["src"], e["dst"]) for e in enc])
    for e, pi in zip(enc, pis):
        e["src"] = pi[e["src"]]
        e["dst"] = pi[e["dst"]]
        e["pi"] = pi

    per_core_edges = []
    for e in enc:
        for r in range(R):
            per_core_edges.append(_rank_edges(cfg, e["src"], e["dst"], r))
    sched, pc_arrays = _build_schedule(cfg, per_core_edges)

    iota = np.broadcast_to(np.arange(cfg["SUPW"] * 128, dtype=np.float16),
                           (128, cfg["SUPW"] * 128)).copy()
    iotaG = np.broadcast_to(np.arange(G, dtype=np.float16), (128, G)).copy()
    gfT = np.ascontiguousarray(np.asarray(inputs["global_feats"], np.float32).T)
    mW0 = np.asarray(inputs["mlp_W0"], np.float32)
    icnts = []
    for e in enc:
        cnt = np.maximum(np.bincount(e["gid"], minlength=G), 1.0).astype(np.float32)
        icnts.append(np.broadcast_to(1.0 / cnt, (128, G)).copy())
    b2_const = float(np.asarray(inputs["mlp_b2"]).reshape(-1)[0])

    xTs, gidrows, nsrcms, vndsts = [], [], [], []
    for e in enc:
        ip = e["pi"][:N]
        xp = np.zeros((NTOT, DIN), np.float16)
        xp[ip] = e["x"].astype(np.float16)
        xTs.append(np.ascontiguousarray(xp.T))
        gr = np.full(NTOT, -1.0, np.float16)
        gr[ip] = e["gid"].astype(np.float16)
        gidrows.append(gr)
        nsp = np.zeros(NTOT, np.float32)
        nsp[ip] = e["nsrc"]
        nsrcms.append(_mat128(nsp))
        ndp = np.zeros(NTOT, np.float16)
        ndp[ip] = e["ndst"].astype(np.float16)
        vndsts.append(ndp)

    in_maps = []
    for gi in range(2):
        pre = "su" if gi == 0 else "sv"
        for r in range(R):
            c = gi * R + r
            arr = pc_arrays[c]
            gr_loc = gidrows[gi][r * NLOC:(r + 1) * NLOC]
            vnd_loc = np.broadcast_to(vndsts[gi][r * NLOC:(r + 1) * NLOC],
                                      (128, NLOC)).copy()
            drm = _mat128(arr["drel"])
            im = dict(
                xT=xTs[gi],
                gidx=_wrap_idx16(arr["gsrc"], HALF),
                drmat=drm.astype(np.float16),
                tohm=(drm[:, :, None] ==
                      np.arange(128, dtype=np.float32)).astype(np.float16),
                iota=iota, iotaG=iotaG,
                gidrow=_mat128(gr_loc),
                nsrcm=nsrcms[gi],
                vndst=vnd_loc,
                W0=np.asarray(inputs[f"{pre}_W0"], np.float32).astype(np.float16),
                W1=np.asarray(inputs[f"{pre}_W1"], np.float32).astype(np.float16),
                W2=np.asarray(inputs[f"{pre}_W2"], np.float32).astype(np.float16),
                bcol=np.ascontiguousarray(
                    np.asarray(inputs[f"{pre}_b"], np.float32).T),
                mW0su=np.ascontiguousarray(mW0[0:DH, :]),
                mW0sv=np.ascontiguousarray(mW0[DH:2 * DH, :]),
                mW0gf=np.ascontiguousarray(mW0[2 * DH:2 * DH + 4, :]),
                mW1=np.asarray(inputs["mlp_W1"], np.float32),
                mW2=np.asarray(inputs["mlp_W2"], np.float32),
                b0c=np.asarray(inputs["mlp_b0"], np.float32).reshape(128, 1),
                b1c=np.asarray(inputs["mlp_b1"], np.float32).reshape(64, 1),
                gfT=gfT, icnt_su=icnts[0], icnt_sv=icnts[1],
            )
            in_maps.append(im)
    return cfg, sched, b2_const, in_maps


ABLATE = frozenset()  # debug: {"scatter","onehot","matmul","ag_local","shared_out"}

_CACHE = {}


def kernel(**inputs) -> np.ndarray:
    cfg, sched, b2c, in_maps = _host_prep(CFG, inputs)
    key = (sched["rows"], b2c, sched["chunk_sw"].tobytes(),
           tuple(i for sw in sched["sw_instrs"] for i in sw))
    nc = _CACHE.get(key)
    if nc is None:
        nc = _build_nc(cfg, sched, b2c, 8)
        _CACHE[key] = nc
    res = run_bass_kernel_spmd(nc, in_maps, core_ids=list(range(8)))
    return np.asarray(res.results[0]["y"], np.float32)
